# revision 32
# baseline (speedup 1.0000x reference)
"""Trainium2 Bass kernel for DynamicGNN (3-layer RGCN-style message passing).

Strategy: shard destination nodes (and their incoming edges) across the 8
NeuronCores. Each core owns N/8 nodes = 3*N/8 (node,relation) segments.
Per-edge messages are gathered from a replicated node-feature table in DRAM
via dma_gather (multi-packet descriptors), segment-reduced with
selection-matrix matmuls on the TensorEngine accumulating both table halves
into one PSUM window, then transformed per-relation and layer-normed in
transposed (feature-major) space via ones-matmul statistics. Node tables are
rebuilt each layer with an AllGather collective; the row-major bounce copy
for the next layer's table is emitted inside the LayerNorm loop.
"""
import sys

import numpy as np

sys.path.insert(0, "/opt/trn_rl_repo")

NCORES = 8
HALF = 32768          # int16 index limit for dma_gather -> split table in 2
WINSEG = 128          # segments per psum window; larger windows reduce the
                      # gather-descriptor pad inflation (1.185 -> 1.06 at 512)
                      # but measured slower (ring starvation vs BUFS) -- 128
                      # is the verified-fastest configuration
BT = 8                # 128-edge tiles per dma_gather call (desc-ring ~1024)
PREFETCH = 4          # gather batches issued ahead of consumption
BUFS = 10             # message-pool ring depth per phase
PSW_BUFS = 4          # PSUM window ring depth (banks)
PSPOST_BUFS = 2       # PSUM post-matmul double buffering (banks)
LN_EPS = 1e-5
NUM_REL = 3

BENCH_ITERS = 10      # kept for test harness compatibility


def _ceil(a, b):
    return (a + b - 1) // b


def _preprocess(x, edge_index, edge_type, sort_src=False, neg_pad=False):
    """Host-side: shard edges by dst owner, sort by segment, build windows,
    A/B phase slot streams, gather indices and per-tile segment metadata."""
    N = x.shape[0]
    E = edge_index.shape[1]
    n_own = N // NCORES
    seg_per_core = n_own * NUM_REL
    nwin = _ceil(seg_per_core, WINSEG)

    src = edge_index[0].astype(np.int64)
    dst = edge_index[1].astype(np.int64)
    et = edge_type.astype(np.int64)

    # global per-(node,rel) counts -> mean denominators
    segg = dst * NUM_REL + et
    counts = np.bincount(segg, minlength=N * NUM_REL).astype(np.float32)
    denom_inv = 1.0 / np.maximum(counts, 1.0)          # [N*R]

    owner = dst // n_own
    cores = []
    cntA = np.zeros((NCORES, nwin), dtype=np.int64)
    cntB = np.zeros((NCORES, nwin), dtype=np.int64)
    per_core = []
    for c in range(NCORES):
        m = owner == c
        s_c = src[m]
        seg_c = (dst[m] - c * n_own) * NUM_REL + et[m]
        order = np.argsort(seg_c, kind="stable")
        s_c = s_c[order]
        seg_c = seg_c[order]
        w_c = seg_c // WINSEG
        isA = s_c < HALF
        cntA[c] = np.bincount(w_c[isA], minlength=nwin)
        cntB[c] = np.bincount(w_c[~isA], minlength=nwin)
        per_core.append((s_c, seg_c, w_c, isA))

    # compile-time tile structure: tiles per (phase, window) = max over cores
    tilesA = _ceil(np.maximum(cntA.max(axis=0), 0), 128)   # [nwin]
    tilesB = _ceil(np.maximum(cntB.max(axis=0), 0), 128)
    TA, TB = int(tilesA.sum()), int(tilesB.sum())
    slotsA, slotsB = TA * 128, TB * 128
    tbaseA = np.concatenate([[0], np.cumsum(tilesA)[:-1]])
    tbaseB = np.concatenate([[0], np.cumsum(tilesB)[:-1]])

    for c in range(NCORES):
        s_c, seg_c, w_c, isA = per_core[c]
        idxA = np.zeros(slotsA, dtype=np.int16)
        idxB = np.zeros(slotsB, dtype=np.int16)
        relA = np.full(slotsA, -1.0, dtype=np.float32)
        relB = np.full(slotsB, -1.0, dtype=np.float32)
        for (mask, idxv, relv, tbase, cnt, off) in (
            (isA, idxA, relA, tbaseA, cntA[c], 0),
            (~isA, idxB, relB, tbaseB, cntB[c], HALF),
        ):
            s_p = s_c[mask] - off
            seg_p = seg_c[mask]
            w_p = w_c[mask]
            if sort_src:
                order2 = np.lexsort((s_p, w_p))
                s_p, seg_p, w_p = s_p[order2], seg_p[order2], w_p[order2]
            gc = np.bincount(w_p, minlength=nwin)
            starts = np.concatenate([[0], np.cumsum(gc)[:-1]])
            pos = np.arange(len(w_p)) - starts[w_p]
            slot = tbase[w_p] * 128 + pos
            if neg_pad:
                idxv[:] = -1
            idxv[slot] = s_p.astype(np.int16)
            relv[slot] = (seg_p - w_p * WINSEG).astype(np.float32)
        cores.append((idxA, idxB, relA, relB))

    segp_pad = _ceil(nwin * WINSEG, 3 * 512) * (3 * 512)
    meta = dict(
        N=N, E=E, n_own=n_own, seg_per_core=seg_per_core, nwin=nwin,
        tilesA=tilesA, tilesB=tilesB, TA=TA, TB=TB, segp_pad=segp_pad,
    )
    return meta, cores, denom_inv


def _pack_idx(flat):
    """[S] int16 -> [128, S/16] with the 16-wrap block replicated across the
    8 gpsimd cores."""
    blk = flat.reshape(-1, 16).T        # [16, S/16]
    return np.tile(blk, (8, 1)).copy()


def _pack_rel(flat):
    """[S] f32 -> [128, S/128]: slot i -> partition i%128, tile i//128."""
    return flat.reshape(-1, 128).T.copy()


def _build_program(meta, single_packet=False, abl=frozenset()):
    # abl (timing attribution only, never set by kernel()):
    #  'gather' - bulk-contiguous DMA of same volume instead of dma_gather
    #  'sel'    - skip sel builds + seg matmuls + drains (Sc memset instead)
    #  'tail'   - skip den scaling, post transform and LN math (keep
    #             transposes/bounce/output writes so the program stays valid)
    #  'coll'   - local DMA copies instead of the AllGather
    import concourse.bacc as bacc
    import concourse.bass as bass
    import concourse.mybir as mybir
    import concourse.tile as tile
    from concourse.masks import make_identity

    dt = mybir.dt
    f32 = dt.float32
    N = meta["N"]
    n_own = meta["n_own"]
    nwin = meta["nwin"]
    tilesA, tilesB = meta["tilesA"], meta["tilesB"]
    TA, TB = meta["TA"], meta["TB"]
    segp_pad = meta["segp_pad"]
    O = 64
    L = 3
    IN_DIM = 5
    rowsA = HALF if N > HALF else N
    rowsB = max(N - HALF, 0)
    NCH = _ceil(n_own, 512)             # post/LN chunks
    WPC = (512 * NUM_REL) // WINSEG     # windows per chunk

    nc = bacc.Bacc("TRN2", target_bir_lowering=False, debug=False,
                   enable_asserts=False, num_devices=NCORES)

    def din(name, shape, d=f32):
        return nc.dram_tensor(name, shape, d, kind="ExternalInput")

    xT_d = din("xT", [IN_DIM, n_own])
    idxA_d = din("idxA", [128, TA * 8], dt.int16)
    idxB_d = din("idxB", [128, max(TB * 8, 16)], dt.int16)
    relA_d = din("relA", [128, TA])
    relB_d = din("relB", [128, max(TB, 1)])
    den_d = din("denInv", [O, segp_pad])
    iota_d = din("iota", [128, WINSEG])
    f2cW_d = din("f2cW", [IN_DIM, O])
    f2cb_d = din("f2cb", [O, 1])
    rgcnW_d = din("rgcnW", [O, L * NUM_REL * O])
    rootW_d = din("rootW", [O, L * O])
    biasT_d = din("biasT", [O, L])
    gamma_d = din("gammaC", [O, 1])
    beta_d = din("betaC", [O, 1])
    out_d = nc.dram_tensor("out", [n_own, O], f32, kind="ExternalOutput")

    AluOp = mybir.AluOpType
    Act = mybir.ActivationFunctionType

    with tile.TileContext(nc) as tc:
        with (
            tc.tile_pool(name="persist", bufs=1) as pp,
            tc.tile_pool(name="msgA", bufs=BUFS) as mpA,
            tc.tile_pool(name="msgB", bufs=BUFS) as mpB,
            tc.tile_pool(name="selp", bufs=6) as selp,
            tc.tile_pool(name="rowp", bufs=6) as rowp,
            tc.tile_pool(name="lnp", bufs=2) as lnp,
            tc.tile_pool(name="scp", bufs=2) as scp,
            tc.tile_pool(name="strp", bufs=3) as strp,
            tc.tile_pool(name="psw", bufs=PSW_BUFS, space="PSUM") as psw,
            tc.tile_pool(name="pspost", bufs=PSPOST_BUFS, space="PSUM") as pspost,
            tc.tile_pool(name="psln", bufs=1, space="PSUM") as psln,
            tc.tile_pool(name="pstr", bufs=1, space="PSUM") as pstr,
            tc.tile_pool(name="dram", bufs=1, space="DRAM") as dr,
        ):
            def persist(name, shape, d=f32):
                return pp.tile(shape, d, tag=name, name=name)

            idxA = persist("idxA", [128, TA * 8], dt.int16)
            idxB = persist("idxB", [128, max(TB * 8, 16)], dt.int16)
            relA = persist("relA", [128, TA])
            relB = persist("relB", [128, max(TB, 1)])
            iota = persist("iota", [128, WINSEG])
            ident = persist("ident", [128, 128])
            f2cW = persist("f2cW", [IN_DIM, O])
            f2cb = persist("f2cb", [O, 1])
            rgcnW = persist("rgcnW", [O, L * NUM_REL * O])
            rootW = persist("rootW", [O, L * O])
            biasT = persist("biasT", [O, L])
            gammaC = persist("gammaC", [O, 1])
            betaC = persist("betaC", [O, 1])
            hT = persist("hT", [O, n_own])
            Jdiv = persist("Jdiv", [O, O])
            ones1 = persist("ones1", [1, O])
            onesC = persist("onesC", [O, 1])
            eps1 = persist("eps1", [1, 1])

            for sb_t, d_t in ((idxA, idxA_d), (idxB, idxB_d),
                              (relA, relA_d), (relB, relB_d),
                              (iota, iota_d), (f2cW, f2cW_d), (f2cb, f2cb_d),
                              (rgcnW, rgcnW_d), (rootW, rootW_d),
                              (biasT, biasT_d), (gammaC, gamma_d),
                              (betaC, beta_d)):
                nc.sync.dma_start(sb_t[:], d_t[:])
            make_identity(nc, ident[:])
            nc.vector.memset(Jdiv[:], 1.0 / O)
            nc.vector.memset(ones1[:], 1.0)
            nc.vector.memset(onesC[:], 1.0)
            nc.vector.memset(eps1[:], LN_EPS)

            # pre-zero message rings so first-use pad slots hold finite zeros
            for pool, tg in ((mpA, "msgA"), (mpB, "msgB")):
                for _ in range(BUFS):
                    mb = pool.tile([128, BT, O], f32, tag=tg, name="mbz")
                    nc.vector.memset(mb[:], 0.0)

            bounce = [dr.tile([n_own, O], f32, tag=f"bounce{l}",
                              name=f"bounce{l}") for l in range(L)]
            table = [nc.dram_tensor(f"table{l}", [N, O], f32, kind="Internal",
                                    addr_space="Shared") for l in range(L)]

            def chunks(total, step):
                return [(i, min(step, total - i)) for i in range(0, total, step)]

            def emit_rows(src_ap, n, l_next, o):
                """transpose [64, n] feature-major chunk at col offset o into
                row-major bounce[l_next] (n <= 512)."""
                for (o2, n2) in chunks(n, 128):
                    ps2 = pstr.tile([128, O], f32, tag="pstr", name="ps2")
                    nc.tensor.matmul(ps2[:n2, :], src_ap[:, o2:o2 + n2],
                                     ident[:O, :O], start=True, stop=True)
                    rows = rowp.tile([128, O], f32, tag="rows", name="rows")
                    nc.scalar.activation(rows[:n2, :], ps2[:n2, :], Act.Copy)
                    nc.sync.dma_start(bounce[l_next][o + o2:o + o2 + n2, :],
                                     rows[:n2, :])

            # ---- layer 0 features + bounce[0] rows ----
            for (o, n) in chunks(n_own, 512):
                xTc = strp.tile([IN_DIM, 512], f32, tag="xTc", name="xTc")
                nc.sync.dma_start(xTc[:, :n], xT_d[:, o:o + n])
                ps = pspost.tile([O, 512], f32, tag="pspost", name="ps")
                nc.tensor.matmul(ps[:, :n], f2cW[:], xTc[:, :n],
                                 start=True, stop=True)
                nc.scalar.activation(hT[:, o:o + n], ps[:, :n], Act.Identity,
                                     bias=f2cb[:])
                emit_rows(hT[:, o:o + n], n, 0, o)

            for l in range(L):
                if "noag" in abl:
                    pass
                elif "coll" in abl:
                    for cc in range(NCORES):
                        nc.sync.dma_start(
                            table[l][cc * n_own:(cc + 1) * n_own, :],
                            bounce[l][:])
                else:
                    nc.gpsimd.collective_compute(
                        "AllGather", AluOp.bypass,
                        replica_groups=[list(range(NCORES))],
                        ins=[bounce[l][:].opt()],
                        outs=[table[l][:].opt()],
                    )

                # ---- merged seg phase + interleaved post/LN ----
                nbA, nbB = _ceil(TA, BT), _ceil(TB, BT)
                msgs = {0: [], 1: []}
                phase_cfg = {
                    0: (TA, idxA, 0, rowsA, mpA, "msgA"),
                    1: (TB, idxB, HALF, rowsB, mpB, "msgB"),
                }

                def ensure(p, b_up, nb):
                    T_p, idx_p, lo, nrows, pool, tg = phase_cfg[p]
                    lst = msgs[p]
                    while len(lst) <= min(b_up, nb - 1):
                        b = len(lst)
                        t0b = b * BT
                        bt = min(BT, T_p - t0b)
                        mb = pool.tile([128, BT, O], f32, tag=tg, name="mb")
                        if "gather" in abl:
                            nc.sync.dma_start(
                                mb[:, :bt, :],
                                table[l][0:128 * bt, :].rearrange(
                                    "(p t) o -> p t o", p=128))
                        else:
                            nc.gpsimd.dma_gather(
                                mb[:, :bt, :],
                                table[l][lo:lo + nrows, :],
                                idx_p[:, t0b * 8: t0b * 8 + bt * 8],
                                bt * 128, bt * 128, O,
                                single_packet=single_packet,
                            )
                        lst.append(mb)

                tiA = tiB = 0
                for c in range(NCH):
                    w0, w1 = WPC * c, min(WPC * (c + 1), nwin)
                    ncols = (w1 - w0) * WINSEG
                    Sc = scp.tile([O, WPC * WINSEG], f32, tag="Sc", name="Sc")
                    denc = strp.tile([O, WPC * WINSEG], f32, tag="denc",
                                     name="denc")
                    nc.sync.dma_start(
                        denc[:, :ncols],
                        den_d[:, w0 * WINSEG:w0 * WINSEG + ncols])
                    for w in range(w0, w1):
                        ntA, ntB = int(tilesA[w]), int(tilesB[w])
                        ensure(0, (tiA + ntA - 1) // BT + PREFETCH, nbA)
                        ensure(1, (tiB + ntB - 1) // BT + PREFETCH, nbB)
                        tot = ntA + ntB
                        col = (w - w0) * WINSEG
                        if tot == 0 or "sel" in abl:
                            nc.vector.memset(Sc[:, col:col + WINSEG], 0.0)
                            tiA += ntA
                            tiB += ntB
                            continue
                        ps = psw.tile([O, WINSEG], f32, tag="psw", name="ps")
                        k = 0
                        for p, ti, nt, rel_p in ((0, tiA, ntA, relA),
                                                 (1, tiB, ntB, relB)):
                            for j in range(nt):
                                t = ti + j
                                sel = selp.tile([128, WINSEG], f32, tag="sel",
                                                name="sel")
                                nc.vector.tensor_scalar(
                                    out=sel[:], in0=iota[:],
                                    scalar1=rel_p[:, t:t + 1], scalar2=None,
                                    op0=AluOp.is_equal,
                                )
                                mb = msgs[p][t // BT]
                                nc.tensor.matmul(
                                    ps[:], mb[:, t - (t // BT) * BT, :], sel[:],
                                    start=(k == 0), stop=(k == tot - 1),
                                )
                                k += 1
                        tiA += ntA
                        tiB += ntB
                        nc.scalar.activation(Sc[:, col:col + WINSEG], ps[:],
                                             Act.Copy)
                    o = 512 * c
                    n = min(512, n_own - o)
                    if "tail" in abl:
                        if l == L - 1:
                            for (o2, n2) in chunks(n, 128):
                                ps2 = pstr.tile([128, O], f32, tag="pstr",
                                                name="ps2")
                                nc.tensor.matmul(ps2[:n2, :],
                                                 hT[:, o + o2:o + o2 + n2],
                                                 ident[:O, :O], start=True,
                                                 stop=True)
                                rows = rowp.tile([128, O], f32, tag="rows",
                                                 name="rows")
                                nc.scalar.activation(rows[:n2, :],
                                                     ps2[:n2, :], Act.Copy)
                                nc.sync.dma_start(
                                    out_d[o + o2:o + o2 + n2, :],
                                    rows[:n2, :])
                        else:
                            emit_rows(hT[:, o:o + n], n, l + 1, o)
                        continue
                    # mean scaling for this chunk
                    nc.vector.tensor_tensor(out=Sc[:, :ncols],
                                            in0=Sc[:, :ncols],
                                            in1=denc[:, :ncols], op=AluOp.mult)

                    # ---- post: relation transform + root + bias + relu ----
                    S_nr = Sc[:, :3 * n].rearrange("p (n r) -> p n r",
                                                   r=NUM_REL)
                    ps = pspost.tile([O, 512], f32, tag="pspost", name="ps")
                    for r in range(NUM_REL):
                        ci = (l * NUM_REL + r) * O
                        nc.tensor.matmul(ps[:, :n], rgcnW[:, ci:ci + O],
                                         S_nr[:, :n, r],
                                         start=(r == 0), stop=False)
                    nc.tensor.matmul(ps[:, :n], rootW[:, l * O:(l + 1) * O],
                                     hT[:, o:o + n], start=False, stop=True)
                    outTc = lnp.tile([O, 512], f32, tag="outTc", name="outTc")
                    nc.scalar.activation(outTc[:, :n], ps[:, :n], Act.Relu,
                                         bias=biasT[:, l:l + 1])

                    # ---- transposed LayerNorm ----
                    mu = psln.tile([O, 512], f32, tag="psln", name="mu")
                    nc.tensor.matmul(mu[:, :n], Jdiv[:], outTc[:, :n],
                                     start=True, stop=True)
                    xc = lnp.tile([O, 512], f32, tag="xc", name="xc")
                    nc.vector.tensor_tensor(out=xc[:, :n], in0=outTc[:, :n],
                                            in1=mu[:, :n], op=AluOp.subtract)
                    sq = lnp.tile([O, 512], f32, tag="sq", name="sq")
                    nc.scalar.activation(sq[:, :n], xc[:, :n], Act.Square)
                    v = psln.tile([O, 512], f32, tag="psln", name="v")
                    nc.tensor.matmul(v[:1, :n], onesC[:], sq[:, :n],
                                     start=True, stop=True)
                    std = lnp.tile([1, 512], f32, tag="std", name="std")
                    nc.scalar.activation(std[:, :n], v[:1, :n], Act.Sqrt,
                                         scale=1.0 / O, bias=eps1[:])
                    rstd = lnp.tile([1, 512], f32, tag="rstd", name="rstd")
                    nc.vector.reciprocal(rstd[:, :n], std[:, :n])
                    rb = psln.tile([O, 512], f32, tag="psln", name="rb")
                    nc.tensor.matmul(rb[:, :n], ones1[:], rstd[:, :n],
                                     start=True, stop=True)
                    t1 = lnp.tile([O, 512], f32, tag="t1", name="t1")
                    nc.vector.scalar_tensor_tensor(
                        out=t1[:, :n], in0=xc[:, :n], scalar=gammaC[:],
                        in1=rb[:, :n], op0=AluOp.mult, op1=AluOp.mult)
                    if l == L - 1:
                        outF = lnp.tile([O, 512], f32, tag="outF", name="outF")
                        nc.scalar.activation(outF[:, :n], t1[:, :n],
                                             Act.Identity, bias=betaC[:])
                        for (o2, n2) in chunks(n, 128):
                            ps2 = pstr.tile([128, O], f32, tag="pstr",
                                            name="ps2")
                            nc.tensor.matmul(ps2[:n2, :], outF[:, o2:o2 + n2],
                                             ident[:O, :O], start=True,
                                             stop=True)
                            rows = rowp.tile([128, O], f32, tag="rows",
                                             name="rows")
                            nc.scalar.activation(rows[:n2, :], ps2[:n2, :],
                                                 Act.Copy)
                            nc.sync.dma_start(out_d[o + o2:o + o2 + n2, :],
                                             rows[:n2, :])
                    else:
                        nc.scalar.activation(hT[:, o:o + n], t1[:, :n],
                                             Act.Identity, bias=betaC[:])
                        emit_rows(hT[:, o:o + n], n, l + 1, o)

    nc.compile()
    return nc


def _build_runner(nc):
    """Persistent jit callable over the 8-core mesh (mirrors
    bass2jax.run_bass_via_pjrt, but reusable across calls and without
    output-buffer donation — the kernel writes every output element)."""
    import jax
    from jax.sharding import Mesh, NamedSharding, PartitionSpec

    import warnings
    with warnings.catch_warnings():
        warnings.simplefilter("ignore")
        from jax.experimental.shard_map import shard_map

    from concourse import bass2jax as b2j
    from concourse import mybir

    b2j.install_neuronx_cc_hook()

    partition_name = (nc.partition_id_tensor.name
                      if nc.partition_id_tensor else None)
    in_names, out_names, out_avals, zero_outs = [], [], [], []
    for alloc in nc.m.functions[0].allocations:
        if not isinstance(alloc, mybir.MemoryLocationSet):
            continue
        name = alloc.memorylocations[0].name
        if alloc.kind == "ExternalInput":
            if name != partition_name:
                in_names.append(name)
        elif alloc.kind == "ExternalOutput":
            shape = tuple(alloc.tensor_shape)
            dtype = mybir.dt.np(alloc.dtype)
            out_names.append(name)
            out_avals.append(jax.core.ShapedArray(shape, dtype))
            zero_outs.append(np.zeros(shape, dtype))
    n_params = len(in_names)
    n_outs = len(out_avals)
    in_names_all = list(in_names) + out_names
    if partition_name is not None:
        in_names_all.append(partition_name)

    def _body(*args):
        operands = list(args)
        if partition_name is not None:
            operands.append(b2j.partition_id_tensor())
        outs = b2j._bass_exec_p.bind(
            *operands, out_avals=tuple(out_avals),
            in_names=tuple(in_names_all), out_names=tuple(out_names),
            lowering_input_output_aliases=(),
            sim_require_finite=True, sim_require_nnan=True, nc=nc)
        return tuple(outs)

    devices = jax.devices()[:NCORES]
    mesh = Mesh(np.asarray(devices), ("core",))
    in_specs = (PartitionSpec("core"),) * (n_params + n_outs)
    out_specs = (PartitionSpec("core"),) * n_outs
    fn = jax.jit(shard_map(_body, mesh=mesh, in_specs=in_specs,
                           out_specs=out_specs, check_rep=False),
                 keep_unused=True)
    sh = NamedSharding(mesh, PartitionSpec("core"))
    return fn, in_names, out_names, out_avals, zero_outs, sh


def _make_in_maps(inputs, meta, cores, denom_inv):
    x = np.asarray(inputs["x"], dtype=np.float32)
    N = x.shape[0]
    n_own = N // NCORES
    O = 64
    L = np.asarray(inputs["rgcn_W"]).shape[0]
    TB = meta["TB"]
    segp_pad = meta["segp_pad"]

    iota = np.broadcast_to(np.arange(WINSEG, dtype=np.float32),
                       (128, WINSEG)).copy()
    f2cW = np.asarray(inputs["feat2c_W"], dtype=np.float32)
    f2cb = np.asarray(inputs["feat2c_b"], dtype=np.float32).reshape(O, 1)
    rgcnW = np.asarray(inputs["rgcn_W"], np.float32).transpose(2, 0, 1, 3)\
        .reshape(O, L * NUM_REL * O)
    rootW = np.asarray(inputs["rgcn_root"], np.float32).transpose(1, 0, 2)\
        .reshape(O, L * O)
    biasT = np.asarray(inputs["rgcn_bias"], np.float32).T.copy()
    gammaC = np.asarray(inputs["ln_gamma"], np.float32).reshape(O, 1)
    betaC = np.asarray(inputs["ln_beta"], np.float32).reshape(O, 1)

    in_maps = []
    for c in range(NCORES):
        idxA, idxB, relA, relB = cores[c]
        den_c = denom_inv[c * n_own * NUM_REL:(c + 1) * n_own * NUM_REL]
        den64 = np.ones((O, segp_pad), dtype=np.float32)
        den64[:, :den_c.size] = den_c
        in_maps.append({
            "xT": x[c * n_own:(c + 1) * n_own, :].T.copy(),
            "idxA": _pack_idx(idxA),
            "idxB": _pack_idx(idxB) if TB > 0 else np.zeros((128, 16), np.int16),
            "relA": _pack_rel(relA),
            "relB": _pack_rel(relB) if TB > 0 else np.full((128, 1), -1.0,
                                                           np.float32),
            "denInv": den64,
            "iota": iota,
            "f2cW": f2cW, "f2cb": f2cb, "rgcnW": rgcnW, "rootW": rootW,
            "biasT": biasT, "gammaC": gammaC, "betaC": betaC,
        })
    return in_maps


class _Session:
    """Compiled program + device-resident inputs; call run() repeatedly."""

    def __init__(self, meta, single_packet=False, abl=frozenset()):
        self.meta = meta
        self.nc = _build_program(meta, single_packet=single_packet, abl=abl)
        (self.fn, self.in_names, self.out_names, self.out_avals,
         self.zero_outs, self.sh) = _build_runner(self.nc)

    def stage(self, in_maps):
        import jax
        concat = [np.concatenate([np.asarray(m[n]) for m in in_maps], axis=0)
                  for n in self.in_names]
        self.dev_in = [jax.device_put(a, self.sh) for a in concat]
        self.dev_z = [jax.device_put(
            np.zeros((NCORES * z.shape[0], *z.shape[1:]), z.dtype), self.sh)
            for z in self.zero_outs]
        jax.block_until_ready(self.dev_in)
        jax.block_until_ready(self.dev_z)

    def run_async(self):
        return self.fn(*self.dev_in, *self.dev_z)

    def run(self):
        import jax
        outs = self.run_async()
        jax.block_until_ready(outs)
        return outs

    def fetch_out(self, outs):
        i = self.out_names.index("out")
        return np.asarray(outs[i])   # [NCORES*n_own, O] == full output


def kernel(x, edge_index, edge_type, feat2c_W, feat2c_b, rgcn_W, rgcn_root,
           rgcn_bias, ln_gamma, ln_beta):
    inputs = dict(x=x, edge_index=edge_index, edge_type=edge_type,
                  feat2c_W=feat2c_W, feat2c_b=feat2c_b, rgcn_W=rgcn_W,
                  rgcn_root=rgcn_root, rgcn_bias=rgcn_bias,
                  ln_gamma=ln_gamma, ln_beta=ln_beta)
    meta, cores, denom_inv = _preprocess(
        np.asarray(x), np.asarray(edge_index), np.asarray(edge_type))
    sess = _Session(meta)
    sess.stage(_make_in_maps(inputs, meta, cores, denom_inv))
    return sess.fetch_out(sess.run())


if __name__ == "__main__":
    pass


# revision 33
# speedup vs baseline: 1.0154x; 1.0154x over previous
"""Trainium2 Bass kernel for DynamicGNN (3-layer RGCN-style message passing).

Strategy: shard destination nodes (and their incoming edges) across the 8
NeuronCores. Each core owns N/8 nodes = 3*N/8 (node,relation) segments.
Per-edge messages are gathered from a replicated node-feature table in DRAM
via dma_gather (multi-packet descriptors), segment-reduced with
selection-matrix matmuls on the TensorEngine accumulating both table halves
into one PSUM window, then transformed per-relation and layer-normed in
transposed (feature-major) space via ones-matmul statistics. Node tables are
rebuilt each layer with an AllGather collective; the row-major bounce copy
for the next layer's table is emitted inside the LayerNorm loop.
"""
import sys

import numpy as np

sys.path.insert(0, "/opt/trn_rl_repo")

NCORES = 8
HALF = 32768          # int16 index limit for dma_gather -> split table in 2
WINSEG = 128          # segments per psum window; larger windows reduce the
                      # gather-descriptor pad inflation (1.185 -> 1.06 at 512)
                      # but measured slower (ring starvation vs BUFS) -- 128
                      # is the verified-fastest configuration
BT = 8                # 128-edge tiles per dma_gather call (desc-ring ~1024)
PREFETCH = 4          # gather batches issued ahead of consumption
BUFS = 10             # message-pool ring depth per phase
PSW_BUFS = 4          # PSUM window ring depth (banks)
PSPOST_BUFS = 2       # PSUM post-matmul double buffering (banks)
DRAIN_DVE = False     # drain PSUM windows on DVE instead of ACT
LN_EPS = 1e-5
NUM_REL = 3

BENCH_ITERS = 10      # kept for test harness compatibility


def _ceil(a, b):
    return (a + b - 1) // b


def _preprocess(x, edge_index, edge_type, sort_src=False, neg_pad=False):
    """Host-side: shard edges by dst owner, sort by segment, build windows,
    A/B phase slot streams, gather indices and per-tile segment metadata."""
    N = x.shape[0]
    E = edge_index.shape[1]
    n_own = N // NCORES
    seg_per_core = n_own * NUM_REL
    nwin = _ceil(seg_per_core, WINSEG)

    src = edge_index[0].astype(np.int64)
    dst = edge_index[1].astype(np.int64)
    et = edge_type.astype(np.int64)

    # global per-(node,rel) counts -> mean denominators
    segg = dst * NUM_REL + et
    counts = np.bincount(segg, minlength=N * NUM_REL).astype(np.float32)
    denom_inv = 1.0 / np.maximum(counts, 1.0)          # [N*R]

    owner = dst // n_own
    cores = []
    cntA = np.zeros((NCORES, nwin), dtype=np.int64)
    cntB = np.zeros((NCORES, nwin), dtype=np.int64)
    per_core = []
    for c in range(NCORES):
        m = owner == c
        s_c = src[m]
        seg_c = (dst[m] - c * n_own) * NUM_REL + et[m]
        order = np.argsort(seg_c, kind="stable")
        s_c = s_c[order]
        seg_c = seg_c[order]
        w_c = seg_c // WINSEG
        isA = s_c < HALF
        cntA[c] = np.bincount(w_c[isA], minlength=nwin)
        cntB[c] = np.bincount(w_c[~isA], minlength=nwin)
        per_core.append((s_c, seg_c, w_c, isA))

    # compile-time tile structure: tiles per (phase, window) = max over cores
    tilesA = _ceil(np.maximum(cntA.max(axis=0), 0), 128)   # [nwin]
    tilesB = _ceil(np.maximum(cntB.max(axis=0), 0), 128)
    TA, TB = int(tilesA.sum()), int(tilesB.sum())
    slotsA, slotsB = TA * 128, TB * 128
    tbaseA = np.concatenate([[0], np.cumsum(tilesA)[:-1]])
    tbaseB = np.concatenate([[0], np.cumsum(tilesB)[:-1]])

    for c in range(NCORES):
        s_c, seg_c, w_c, isA = per_core[c]
        idxA = np.zeros(slotsA, dtype=np.int16)
        idxB = np.zeros(slotsB, dtype=np.int16)
        relA = np.full(slotsA, -1.0, dtype=np.float32)
        relB = np.full(slotsB, -1.0, dtype=np.float32)
        for (mask, idxv, relv, tbase, cnt, off) in (
            (isA, idxA, relA, tbaseA, cntA[c], 0),
            (~isA, idxB, relB, tbaseB, cntB[c], HALF),
        ):
            s_p = s_c[mask] - off
            seg_p = seg_c[mask]
            w_p = w_c[mask]
            if sort_src:
                order2 = np.lexsort((s_p, w_p))
                s_p, seg_p, w_p = s_p[order2], seg_p[order2], w_p[order2]
            gc = np.bincount(w_p, minlength=nwin)
            starts = np.concatenate([[0], np.cumsum(gc)[:-1]])
            pos = np.arange(len(w_p)) - starts[w_p]
            slot = tbase[w_p] * 128 + pos
            if neg_pad:
                idxv[:] = -1
            idxv[slot] = s_p.astype(np.int16)
            relv[slot] = (seg_p - w_p * WINSEG).astype(np.float32)
        cores.append((idxA, idxB, relA, relB))

    segp_pad = _ceil(nwin * WINSEG, 3 * 512) * (3 * 512)
    meta = dict(
        N=N, E=E, n_own=n_own, seg_per_core=seg_per_core, nwin=nwin,
        tilesA=tilesA, tilesB=tilesB, TA=TA, TB=TB, segp_pad=segp_pad,
    )
    return meta, cores, denom_inv


def _pack_idx(flat):
    """[S] int16 -> [128, S/16] with the 16-wrap block replicated across the
    8 gpsimd cores."""
    blk = flat.reshape(-1, 16).T        # [16, S/16]
    return np.tile(blk, (8, 1)).copy()


def _pack_rel(flat):
    """[S] f32 -> [128, S/128]: slot i -> partition i%128, tile i//128."""
    return flat.reshape(-1, 128).T.copy()


def _build_program(meta, single_packet=False, abl=frozenset()):
    # abl (timing attribution only, never set by kernel()):
    #  'gather' - bulk-contiguous DMA of same volume instead of dma_gather
    #  'sel'    - skip sel builds + seg matmuls + drains (Sc memset instead)
    #  'tail'   - skip den scaling, post transform and LN math (keep
    #             transposes/bounce/output writes so the program stays valid)
    #  'coll'   - local DMA copies instead of the AllGather
    import concourse.bacc as bacc
    import concourse.bass as bass
    import concourse.mybir as mybir
    import concourse.tile as tile
    from concourse.masks import make_identity

    dt = mybir.dt
    f32 = dt.float32
    N = meta["N"]
    n_own = meta["n_own"]
    nwin = meta["nwin"]
    tilesA, tilesB = meta["tilesA"], meta["tilesB"]
    TA, TB = meta["TA"], meta["TB"]
    segp_pad = meta["segp_pad"]
    O = 64
    L = 3
    IN_DIM = 5
    rowsA = HALF if N > HALF else N
    rowsB = max(N - HALF, 0)
    NCH = _ceil(n_own, 512)             # post/LN chunks
    WPC = (512 * NUM_REL) // WINSEG     # windows per chunk

    nc = bacc.Bacc("TRN2", target_bir_lowering=False, debug=False,
                   enable_asserts=False, num_devices=NCORES)

    def din(name, shape, d=f32):
        return nc.dram_tensor(name, shape, d, kind="ExternalInput")

    xT_d = din("xT", [IN_DIM, n_own])
    idxA_d = din("idxA", [128, TA * 8], dt.int16)
    idxB_d = din("idxB", [128, max(TB * 8, 16)], dt.int16)
    relA_d = din("relA", [128, TA])
    relB_d = din("relB", [128, max(TB, 1)])
    den_d = din("denInv", [O, segp_pad])
    iota_d = din("iota", [128, WINSEG])
    f2cW_d = din("f2cW", [IN_DIM, O])
    f2cb_d = din("f2cb", [O, 1])
    rgcnW_d = din("rgcnW", [O, L * NUM_REL * O])
    rootW_d = din("rootW", [O, L * O])
    biasT_d = din("biasT", [O, L])
    gamma_d = din("gammaC", [O, 1])
    beta_d = din("betaC", [O, 1])
    out_d = nc.dram_tensor("out", [n_own, O], f32, kind="ExternalOutput")

    AluOp = mybir.AluOpType
    Act = mybir.ActivationFunctionType

    with tile.TileContext(nc) as tc:
        with (
            tc.tile_pool(name="persist", bufs=1) as pp,
            tc.tile_pool(name="msgA", bufs=BUFS) as mpA,
            tc.tile_pool(name="msgB", bufs=BUFS) as mpB,
            tc.tile_pool(name="selp", bufs=6) as selp,
            tc.tile_pool(name="rowp", bufs=6) as rowp,
            tc.tile_pool(name="lnp", bufs=2) as lnp,
            tc.tile_pool(name="scp", bufs=2) as scp,
            tc.tile_pool(name="strp", bufs=3) as strp,
            tc.tile_pool(name="psw", bufs=PSW_BUFS, space="PSUM") as psw,
            tc.tile_pool(name="pspost", bufs=PSPOST_BUFS, space="PSUM") as pspost,
            tc.tile_pool(name="psln", bufs=1, space="PSUM") as psln,
            tc.tile_pool(name="pstr", bufs=1, space="PSUM") as pstr,
            tc.tile_pool(name="dram", bufs=1, space="DRAM") as dr,
        ):
            def persist(name, shape, d=f32):
                return pp.tile(shape, d, tag=name, name=name)

            idxA = persist("idxA", [128, TA * 8], dt.int16)
            idxB = persist("idxB", [128, max(TB * 8, 16)], dt.int16)
            relA = persist("relA", [128, TA])
            relB = persist("relB", [128, max(TB, 1)])
            iota = persist("iota", [128, WINSEG])
            ident = persist("ident", [128, 128])
            f2cW = persist("f2cW", [IN_DIM, O])
            f2cb = persist("f2cb", [O, 1])
            rgcnW = persist("rgcnW", [O, L * NUM_REL * O])
            rootW = persist("rootW", [O, L * O])
            biasT = persist("biasT", [O, L])
            gammaC = persist("gammaC", [O, 1])
            betaC = persist("betaC", [O, 1])
            hT = persist("hT", [O, n_own])
            Jdiv = persist("Jdiv", [O, O])
            ones1 = persist("ones1", [1, O])
            onesC = persist("onesC", [O, 1])
            eps1 = persist("eps1", [1, 1])

            for sb_t, d_t in ((idxA, idxA_d), (idxB, idxB_d),
                              (relA, relA_d), (relB, relB_d),
                              (iota, iota_d), (f2cW, f2cW_d), (f2cb, f2cb_d),
                              (rgcnW, rgcnW_d), (rootW, rootW_d),
                              (biasT, biasT_d), (gammaC, gamma_d),
                              (betaC, beta_d)):
                nc.sync.dma_start(sb_t[:], d_t[:])
            make_identity(nc, ident[:])
            nc.vector.memset(Jdiv[:], 1.0 / O)
            nc.vector.memset(ones1[:], 1.0)
            nc.vector.memset(onesC[:], 1.0)
            nc.vector.memset(eps1[:], LN_EPS)

            # pre-zero message rings so first-use pad slots hold finite zeros
            for pool, tg in ((mpA, "msgA"), (mpB, "msgB")):
                for _ in range(BUFS):
                    mb = pool.tile([128, BT, O], f32, tag=tg, name="mbz")
                    nc.vector.memset(mb[:], 0.0)

            bounce = [dr.tile([n_own, O], f32, tag=f"bounce{l}",
                              name=f"bounce{l}") for l in range(L)]
            table = [nc.dram_tensor(f"table{l}", [N, O], f32, kind="Internal",
                                    addr_space="Shared") for l in range(L)]

            def chunks(total, step):
                return [(i, min(step, total - i)) for i in range(0, total, step)]

            def emit_rows(src_ap, n, l_next, o):
                """transpose [64, n] feature-major chunk at col offset o into
                row-major bounce[l_next] (n <= 512)."""
                for (o2, n2) in chunks(n, 128):
                    ps2 = pstr.tile([128, O], f32, tag="pstr", name="ps2")
                    nc.tensor.matmul(ps2[:n2, :], src_ap[:, o2:o2 + n2],
                                     ident[:O, :O], start=True, stop=True)
                    rows = rowp.tile([128, O], f32, tag="rows", name="rows")
                    nc.scalar.activation(rows[:n2, :], ps2[:n2, :], Act.Copy)
                    nc.sync.dma_start(bounce[l_next][o + o2:o + o2 + n2, :],
                                     rows[:n2, :])

            # ---- layer 0 features + bounce[0] rows ----
            for (o, n) in chunks(n_own, 512):
                xTc = strp.tile([IN_DIM, 512], f32, tag="xTc", name="xTc")
                nc.sync.dma_start(xTc[:, :n], xT_d[:, o:o + n])
                ps = pspost.tile([O, 512], f32, tag="pspost", name="ps")
                nc.tensor.matmul(ps[:, :n], f2cW[:], xTc[:, :n],
                                 start=True, stop=True)
                nc.scalar.activation(hT[:, o:o + n], ps[:, :n], Act.Identity,
                                     bias=f2cb[:])
                emit_rows(hT[:, o:o + n], n, 0, o)

            for l in range(L):
                if "noag" in abl:
                    pass
                elif "coll" in abl:
                    for cc in range(NCORES):
                        nc.sync.dma_start(
                            table[l][cc * n_own:(cc + 1) * n_own, :],
                            bounce[l][:])
                else:
                    nc.gpsimd.collective_compute(
                        "AllGather", AluOp.bypass,
                        replica_groups=[list(range(NCORES))],
                        ins=[bounce[l][:].opt()],
                        outs=[table[l][:].opt()],
                    )

                # ---- merged seg phase + interleaved post/LN ----
                nbA, nbB = _ceil(TA, BT), _ceil(TB, BT)
                msgs = {0: [], 1: []}
                phase_cfg = {
                    0: (TA, idxA, 0, rowsA, mpA, "msgA"),
                    1: (TB, idxB, HALF, rowsB, mpB, "msgB"),
                }

                def ensure(p, b_up, nb):
                    T_p, idx_p, lo, nrows, pool, tg = phase_cfg[p]
                    lst = msgs[p]
                    while len(lst) <= min(b_up, nb - 1):
                        b = len(lst)
                        t0b = b * BT
                        bt = min(BT, T_p - t0b)
                        mb = pool.tile([128, BT, O], f32, tag=tg, name="mb")
                        if "gather" in abl:
                            nc.sync.dma_start(
                                mb[:, :bt, :],
                                table[l][0:128 * bt, :].rearrange(
                                    "(p t) o -> p t o", p=128))
                        else:
                            nc.gpsimd.dma_gather(
                                mb[:, :bt, :],
                                table[l][lo:lo + nrows, :],
                                idx_p[:, t0b * 8: t0b * 8 + bt * 8],
                                bt * 128, bt * 128, O,
                                single_packet=single_packet,
                            )
                        lst.append(mb)

                tiA = tiB = 0
                for c in range(NCH):
                    w0, w1 = WPC * c, min(WPC * (c + 1), nwin)
                    ncols = (w1 - w0) * WINSEG
                    Sc = scp.tile([O, WPC * WINSEG], f32, tag="Sc", name="Sc")
                    denc = strp.tile([O, WPC * WINSEG], f32, tag="denc",
                                     name="denc")
                    nc.sync.dma_start(
                        denc[:, :ncols],
                        den_d[:, w0 * WINSEG:w0 * WINSEG + ncols])
                    for w in range(w0, w1):
                        ntA, ntB = int(tilesA[w]), int(tilesB[w])
                        ensure(0, (tiA + ntA - 1) // BT + PREFETCH, nbA)
                        ensure(1, (tiB + ntB - 1) // BT + PREFETCH, nbB)
                        tot = ntA + ntB
                        col = (w - w0) * WINSEG
                        if tot == 0 or "sel" in abl:
                            nc.vector.memset(Sc[:, col:col + WINSEG], 0.0)
                            tiA += ntA
                            tiB += ntB
                            continue
                        ps = psw.tile([O, WINSEG], f32, tag="psw", name="ps")
                        k = 0
                        for p, ti, nt, rel_p in ((0, tiA, ntA, relA),
                                                 (1, tiB, ntB, relB)):
                            for j in range(nt):
                                t = ti + j
                                sel = selp.tile([128, WINSEG], f32, tag="sel",
                                                name="sel")
                                nc.vector.tensor_scalar(
                                    out=sel[:], in0=iota[:],
                                    scalar1=rel_p[:, t:t + 1], scalar2=None,
                                    op0=AluOp.is_equal,
                                )
                                mb = msgs[p][t // BT]
                                nc.tensor.matmul(
                                    ps[:], mb[:, t - (t // BT) * BT, :], sel[:],
                                    start=(k == 0), stop=(k == tot - 1),
                                )
                                k += 1
                        tiA += ntA
                        tiB += ntB
                        if DRAIN_DVE:
                            nc.vector.tensor_scalar(
                                out=Sc[:, col:col + WINSEG], in0=ps[:],
                                scalar1=0.0, scalar2=None, op0=AluOp.add)
                        else:
                            nc.scalar.activation(Sc[:, col:col + WINSEG],
                                                 ps[:], Act.Copy)
                    o = 512 * c
                    n = min(512, n_own - o)
                    if "tail" in abl:
                        if l == L - 1:
                            for (o2, n2) in chunks(n, 128):
                                ps2 = pstr.tile([128, O], f32, tag="pstr",
                                                name="ps2")
                                nc.tensor.matmul(ps2[:n2, :],
                                                 hT[:, o + o2:o + o2 + n2],
                                                 ident[:O, :O], start=True,
                                                 stop=True)
                                rows = rowp.tile([128, O], f32, tag="rows",
                                                 name="rows")
                                nc.scalar.activation(rows[:n2, :],
                                                     ps2[:n2, :], Act.Copy)
                                nc.sync.dma_start(
                                    out_d[o + o2:o + o2 + n2, :],
                                    rows[:n2, :])
                        else:
                            emit_rows(hT[:, o:o + n], n, l + 1, o)
                        continue
                    # mean scaling for this chunk
                    nc.vector.tensor_tensor(out=Sc[:, :ncols],
                                            in0=Sc[:, :ncols],
                                            in1=denc[:, :ncols], op=AluOp.mult)

                    # ---- post: relation transform + root + bias + relu ----
                    S_nr = Sc[:, :3 * n].rearrange("p (n r) -> p n r",
                                                   r=NUM_REL)
                    ps = pspost.tile([O, 512], f32, tag="pspost", name="ps")
                    for r in range(NUM_REL):
                        ci = (l * NUM_REL + r) * O
                        nc.tensor.matmul(ps[:, :n], rgcnW[:, ci:ci + O],
                                         S_nr[:, :n, r],
                                         start=(r == 0), stop=False)
                    nc.tensor.matmul(ps[:, :n], rootW[:, l * O:(l + 1) * O],
                                     hT[:, o:o + n], start=False, stop=True)
                    outTc = lnp.tile([O, 512], f32, tag="outTc", name="outTc")
                    nc.scalar.activation(outTc[:, :n], ps[:, :n], Act.Relu,
                                         bias=biasT[:, l:l + 1])

                    # ---- transposed LayerNorm ----
                    mu = psln.tile([O, 512], f32, tag="psln", name="mu")
                    nc.tensor.matmul(mu[:, :n], Jdiv[:], outTc[:, :n],
                                     start=True, stop=True)
                    xc = lnp.tile([O, 512], f32, tag="xc", name="xc")
                    nc.vector.tensor_tensor(out=xc[:, :n], in0=outTc[:, :n],
                                            in1=mu[:, :n], op=AluOp.subtract)
                    sq = lnp.tile([O, 512], f32, tag="sq", name="sq")
                    nc.scalar.activation(sq[:, :n], xc[:, :n], Act.Square)
                    v = psln.tile([O, 512], f32, tag="psln", name="v")
                    nc.tensor.matmul(v[:1, :n], onesC[:], sq[:, :n],
                                     start=True, stop=True)
                    std = lnp.tile([1, 512], f32, tag="std", name="std")
                    nc.scalar.activation(std[:, :n], v[:1, :n], Act.Sqrt,
                                         scale=1.0 / O, bias=eps1[:])
                    rstd = lnp.tile([1, 512], f32, tag="rstd", name="rstd")
                    nc.vector.reciprocal(rstd[:, :n], std[:, :n])
                    rb = psln.tile([O, 512], f32, tag="psln", name="rb")
                    nc.tensor.matmul(rb[:, :n], ones1[:], rstd[:, :n],
                                     start=True, stop=True)
                    t1 = lnp.tile([O, 512], f32, tag="t1", name="t1")
                    nc.vector.scalar_tensor_tensor(
                        out=t1[:, :n], in0=xc[:, :n], scalar=gammaC[:],
                        in1=rb[:, :n], op0=AluOp.mult, op1=AluOp.mult)
                    if l == L - 1:
                        outF = lnp.tile([O, 512], f32, tag="outF", name="outF")
                        nc.scalar.activation(outF[:, :n], t1[:, :n],
                                             Act.Identity, bias=betaC[:])
                        for (o2, n2) in chunks(n, 128):
                            ps2 = pstr.tile([128, O], f32, tag="pstr",
                                            name="ps2")
                            nc.tensor.matmul(ps2[:n2, :], outF[:, o2:o2 + n2],
                                             ident[:O, :O], start=True,
                                             stop=True)
                            rows = rowp.tile([128, O], f32, tag="rows",
                                             name="rows")
                            nc.scalar.activation(rows[:n2, :], ps2[:n2, :],
                                                 Act.Copy)
                            nc.sync.dma_start(out_d[o + o2:o + o2 + n2, :],
                                             rows[:n2, :])
                    else:
                        nc.scalar.activation(hT[:, o:o + n], t1[:, :n],
                                             Act.Identity, bias=betaC[:])
                        emit_rows(hT[:, o:o + n], n, l + 1, o)

    nc.compile()
    return nc


def _build_runner(nc):
    """Persistent jit callable over the 8-core mesh (mirrors
    bass2jax.run_bass_via_pjrt, but reusable across calls and without
    output-buffer donation — the kernel writes every output element)."""
    import jax
    from jax.sharding import Mesh, NamedSharding, PartitionSpec

    import warnings
    with warnings.catch_warnings():
        warnings.simplefilter("ignore")
        from jax.experimental.shard_map import shard_map

    from concourse import bass2jax as b2j
    from concourse import mybir

    b2j.install_neuronx_cc_hook()

    partition_name = (nc.partition_id_tensor.name
                      if nc.partition_id_tensor else None)
    in_names, out_names, out_avals, zero_outs = [], [], [], []
    for alloc in nc.m.functions[0].allocations:
        if not isinstance(alloc, mybir.MemoryLocationSet):
            continue
        name = alloc.memorylocations[0].name
        if alloc.kind == "ExternalInput":
            if name != partition_name:
                in_names.append(name)
        elif alloc.kind == "ExternalOutput":
            shape = tuple(alloc.tensor_shape)
            dtype = mybir.dt.np(alloc.dtype)
            out_names.append(name)
            out_avals.append(jax.core.ShapedArray(shape, dtype))
            zero_outs.append(np.zeros(shape, dtype))
    n_params = len(in_names)
    n_outs = len(out_avals)
    in_names_all = list(in_names) + out_names
    if partition_name is not None:
        in_names_all.append(partition_name)

    def _body(*args):
        operands = list(args)
        if partition_name is not None:
            operands.append(b2j.partition_id_tensor())
        outs = b2j._bass_exec_p.bind(
            *operands, out_avals=tuple(out_avals),
            in_names=tuple(in_names_all), out_names=tuple(out_names),
            lowering_input_output_aliases=(),
            sim_require_finite=True, sim_require_nnan=True, nc=nc)
        return tuple(outs)

    devices = jax.devices()[:NCORES]
    mesh = Mesh(np.asarray(devices), ("core",))
    in_specs = (PartitionSpec("core"),) * (n_params + n_outs)
    out_specs = (PartitionSpec("core"),) * n_outs
    fn = jax.jit(shard_map(_body, mesh=mesh, in_specs=in_specs,
                           out_specs=out_specs, check_rep=False),
                 keep_unused=True)
    sh = NamedSharding(mesh, PartitionSpec("core"))
    return fn, in_names, out_names, out_avals, zero_outs, sh


def _make_in_maps(inputs, meta, cores, denom_inv):
    x = np.asarray(inputs["x"], dtype=np.float32)
    N = x.shape[0]
    n_own = N // NCORES
    O = 64
    L = np.asarray(inputs["rgcn_W"]).shape[0]
    TB = meta["TB"]
    segp_pad = meta["segp_pad"]

    iota = np.broadcast_to(np.arange(WINSEG, dtype=np.float32),
                       (128, WINSEG)).copy()
    f2cW = np.asarray(inputs["feat2c_W"], dtype=np.float32)
    f2cb = np.asarray(inputs["feat2c_b"], dtype=np.float32).reshape(O, 1)
    rgcnW = np.asarray(inputs["rgcn_W"], np.float32).transpose(2, 0, 1, 3)\
        .reshape(O, L * NUM_REL * O)
    rootW = np.asarray(inputs["rgcn_root"], np.float32).transpose(1, 0, 2)\
        .reshape(O, L * O)
    biasT = np.asarray(inputs["rgcn_bias"], np.float32).T.copy()
    gammaC = np.asarray(inputs["ln_gamma"], np.float32).reshape(O, 1)
    betaC = np.asarray(inputs["ln_beta"], np.float32).reshape(O, 1)

    in_maps = []
    for c in range(NCORES):
        idxA, idxB, relA, relB = cores[c]
        den_c = denom_inv[c * n_own * NUM_REL:(c + 1) * n_own * NUM_REL]
        den64 = np.ones((O, segp_pad), dtype=np.float32)
        den64[:, :den_c.size] = den_c
        in_maps.append({
            "xT": x[c * n_own:(c + 1) * n_own, :].T.copy(),
            "idxA": _pack_idx(idxA),
            "idxB": _pack_idx(idxB) if TB > 0 else np.zeros((128, 16), np.int16),
            "relA": _pack_rel(relA),
            "relB": _pack_rel(relB) if TB > 0 else np.full((128, 1), -1.0,
                                                           np.float32),
            "denInv": den64,
            "iota": iota,
            "f2cW": f2cW, "f2cb": f2cb, "rgcnW": rgcnW, "rootW": rootW,
            "biasT": biasT, "gammaC": gammaC, "betaC": betaC,
        })
    return in_maps


class _Session:
    """Compiled program + device-resident inputs; call run() repeatedly."""

    def __init__(self, meta, single_packet=False, abl=frozenset()):
        self.meta = meta
        self.nc = _build_program(meta, single_packet=single_packet, abl=abl)
        (self.fn, self.in_names, self.out_names, self.out_avals,
         self.zero_outs, self.sh) = _build_runner(self.nc)

    def stage(self, in_maps):
        import jax
        concat = [np.concatenate([np.asarray(m[n]) for m in in_maps], axis=0)
                  for n in self.in_names]
        self.dev_in = [jax.device_put(a, self.sh) for a in concat]
        self.dev_z = [jax.device_put(
            np.zeros((NCORES * z.shape[0], *z.shape[1:]), z.dtype), self.sh)
            for z in self.zero_outs]
        jax.block_until_ready(self.dev_in)
        jax.block_until_ready(self.dev_z)

    def run_async(self):
        return self.fn(*self.dev_in, *self.dev_z)

    def run(self):
        import jax
        outs = self.run_async()
        jax.block_until_ready(outs)
        return outs

    def fetch_out(self, outs):
        i = self.out_names.index("out")
        return np.asarray(outs[i])   # [NCORES*n_own, O] == full output


def kernel(x, edge_index, edge_type, feat2c_W, feat2c_b, rgcn_W, rgcn_root,
           rgcn_bias, ln_gamma, ln_beta):
    inputs = dict(x=x, edge_index=edge_index, edge_type=edge_type,
                  feat2c_W=feat2c_W, feat2c_b=feat2c_b, rgcn_W=rgcn_W,
                  rgcn_root=rgcn_root, rgcn_bias=rgcn_bias,
                  ln_gamma=ln_gamma, ln_beta=ln_beta)
    meta, cores, denom_inv = _preprocess(
        np.asarray(x), np.asarray(edge_index), np.asarray(edge_type))
    sess = _Session(meta)
    sess.stage(_make_in_maps(inputs, meta, cores, denom_inv))
    return sess.fetch_out(sess.run())


if __name__ == "__main__":
    pass
